# revision 22
# baseline (speedup 1.0000x reference)
"""DJMGNN (NNConv/GraphNorm GNN) Trainium2 kernel, 8-core SPMD. v3.

Sharding: nodes range-sharded N/8 per core, then PERMUTED within each shard so
every 128-node dst window holds <=512 edges (uniform 4 blocks/window, B=80).
Edges assigned to the core owning their dst node.

v5 (from v2 profile: span ~710us, Tensor 84%/Vector 72% busy):
  - edge mult paired into one 4-dim 2x-mode DVE TT per block pair (1217ns);
    i-reduction as two 2x tree levels + one small 1x reduce (1778ns/pair,
    was 2277ns single-instruction 1x reduce).
  - root matmul accumulates directly into the scatter PSUM bank (one
    accumulation group per window); root/final bias rows zeroed and folded
    into the GraphNorm C/D rows exactly (general in ms).
  - GraphNorm stats: DVE square + w-reduces + ONE fp32 ones-matmul
    (replaces 20 accumulating PE matmuls per layer). Needs conv pads == 0,
    guaranteed by masked pass-2 relu + zeroed bias rows.
  - transposes moved out of the edge phase into pass 2 (hnxt^T produced
    right after hnxt), freeing PSUM for a 3-deep MLP ring (pre bufs=3) so
    the PE is not chained to the Act relu cadence.
  - next-layer MLP runahead (14 pairs) spread through the stats-AllReduce
    wait and the pass-2 loops.
  NOTE: scatters lag the DVE pipeline by one pair (cross-window), conv
  copies fire when a window's stop-scatter retires. Indirect gathers must
  use single-block [128,1]-offset form: multi-block offset APs and 3-dim
  out APs return wrong data on HW (CoreSim disagrees -- verified).
"""

import sys

if "/opt/trn_rl_repo" not in sys.path:
    sys.path.insert(0, "/opt/trn_rl_repo")

import numpy as np
import ml_dtypes

import concourse.bass as bass
import concourse.bacc as bacc
import concourse.mybir as mybir
import concourse.tile as tile

mdt = mybir.dt
AF = mybir.ActivationFunctionType
ALU = mybir.AluOpType

NCORES = 8
EPS = 1e-5
RUN_PAIRS = 14  # next-layer MLP pairs emitted during stats-AR + pass 2
GB = 1  # gather batch (blocks per indirect DMA); 1 until HW perm resolved
DEBUG = False  # add layer-0 intermediate outputs for debugging


# ---------------------------------------------------------------- host prep


def _balance_windows(deg, nwin, cap_n=128):
    """Assign local nodes to nwin windows, balancing edge load (greedy LPT)."""
    order = np.argsort(-deg, kind="stable")
    wload = np.zeros(nwin, dtype=np.int64)
    wn = np.zeros(nwin, dtype=np.int64)
    assign = np.full(deg.shape[0], -1, dtype=np.int64)
    for v in order:
        cand = np.where(wn < cap_n)[0]
        w = cand[np.argmin(wload[cand])]
        assign[v] = w
        wload[w] += deg[v]
        wn[w] += 1
    return assign, wload, wn


def prep_inputs(inputs):
    x = np.asarray(inputs["x"], np.float32)
    edge_attr = np.asarray(inputs["edge_attr"], np.float32)
    edge_index = np.asarray(inputs["edge_index"])
    N, IN = x.shape
    E, EA = edge_attr.shape
    H = np.asarray(inputs["init_W"]).shape[1]
    L = np.asarray(inputs["edge_mlp_W"]).shape[0]
    T = np.asarray(inputs["final_W"]).shape[1]
    shard = N // NCORES
    nwin = (shard + 127) // 128
    shard_pad = nwin * 128
    src = edge_index[0].astype(np.int64)
    dst = edge_index[1].astype(np.int64)
    owner = dst // shard
    dst_local = dst - owner * shard

    # per-core window assignment (node permutation)
    assigns, wns = [], []
    pos_of_node = np.empty((NCORES, shard), np.int64)  # node -> padded position
    node_at_pos = np.full((NCORES, shard_pad), -1, np.int64)
    cnt = np.zeros((NCORES, nwin), np.int64)
    for c in range(NCORES):
        dl = dst_local[owner == c]
        deg = np.bincount(dl, minlength=shard)
        assign, wload, wn = _balance_windows(deg, nwin)
        assigns.append(assign)
        wns.append(wn)
        cnt[c] = wload
        row_next = np.zeros(nwin, np.int64)
        for v in range(shard):
            w = assign[v]
            p = w * 128 + row_next[w]
            row_next[w] += 1
            pos_of_node[c, v] = p
            node_at_pos[c, p] = v

    bw = np.maximum((cnt + 127) // 128, 1).max(axis=0)  # blocks per window
    block_win = []
    for w in range(nwin):
        block_win += [w] * int(bw[w])
    B = len(block_win)

    # slot assignment: edges of (core, window) packed into that window's blocks
    eslot = np.full((NCORES, B, 128), -1, dtype=np.int64)
    wb0 = np.concatenate([[0], np.cumsum(bw)])  # first block of window w
    for c in range(NCORES):
        ec = np.where(owner == c)[0]
        wids = assigns[c][dst_local[ec]]
        for w in range(nwin):
            es = ec[wids == w]
            b0 = wb0[w]
            for j, e in enumerate(es):
                eslot[c, b0 + j // 128, j % 128] = e

    # host-side h0 (layer-0 table): h0 = x @ init_W + init_b
    h0 = x @ np.asarray(inputs["init_W"], np.float32) + np.asarray(
        inputs["init_b"], np.float32
    )

    # o-major reorder of edge MLP weights: col j = i*H + o -> o*H + i
    Wm = np.asarray(inputs["edge_mlp_W"], np.float32).reshape(L, EA, H, H)
    Wm = Wm.transpose(0, 1, 3, 2).reshape(L, EA, H * H)
    bm = np.asarray(inputs["edge_mlp_b"], np.float32).reshape(L, H, H)
    bm = bm.transpose(0, 2, 1).reshape(L, H * H)
    W_aug = np.concatenate([Wm, bm[:, None, :]], axis=1)  # [L, EA+1, H*H]

    # root weights WITHOUT bias (bias folded into GraphNorm C/D rows)
    rootW = np.asarray(inputs["root_W"], np.float32)
    root_aug = np.zeros((L, 2 * H + 1, H), np.float32)
    root_aug[:, :H, :] = rootW

    trans_aug = np.concatenate(
        [np.asarray(inputs["trans_W"], np.float32),
         np.asarray(inputs["trans_b"], np.float32)[:, None, :]], axis=1
    )  # [L, 2H+1, H]

    # final transition WITHOUT bias (folded into final GraphNorm)
    final_aug = np.zeros((2 * H + 1, T), np.float32)
    final_aug[:H, :] = np.asarray(inputs["final_W"], np.float32)

    # gn row: [w, b, ms, fold_bias] per layer
    gn = np.concatenate(
        [np.asarray(inputs["gn_w"], np.float32),
         np.asarray(inputs["gn_b"], np.float32),
         np.asarray(inputs["gn_ms"], np.float32),
         np.asarray(inputs["root_b"], np.float32)], axis=1
    )[:, None, :]  # [L, 1, 4H]
    fgn = np.concatenate(
        [np.asarray(inputs["fgn_w"], np.float32),
         np.asarray(inputs["fgn_b"], np.float32),
         np.asarray(inputs["fgn_ms"], np.float32),
         np.asarray(inputs["final_b"], np.float32)], axis=0
    )[None, :]  # [1, 4T]

    ident = np.eye(128, dtype=np.float32)
    ones_row = np.ones((1, 128), np.float32)
    ones_col = np.ones((128, 1), np.float32)

    in_maps = []
    for c in range(NCORES):
        es = eslot[c]
        valid = es >= 0
        esc = np.where(valid, es, 0)
        flat = esc.reshape(-1)
        vflat = valid.reshape(-1)

        attrT_aug = np.zeros((EA + 1, B * 128), np.float32)
        attrT_aug[:EA, :] = edge_attr[flat].T * vflat
        attrT_aug[EA, :] = vflat.astype(np.float32)

        sg = src[flat]
        gidx = (sg // shard) * shard_pad + pos_of_node[sg // shard, sg % shard]
        gidx = np.where(vflat, gidx, 0).astype(np.int32)
        src_gidx = gidx.reshape(B, 128).T.copy()

        # pre-gathered layer-0 h[src] (pad slots zeroed)
        hsrc0 = (h0[sg] * vflat[:, None]).reshape(B, 128, H).transpose(1, 0, 2)

        wl = assigns[c][dst_local[flat]]
        dr = np.where(vflat,
                      pos_of_node[c, dst_local[flat]] - wl * 128, -1.0)
        dst_rel = dr.astype(np.float32).reshape(B, 128).T
        # host-built one-hot scatter blocks [128 slot, B, 128 dstrow]
        onehot = (np.arange(128, dtype=np.float32)[None, None, :]
                  == dst_rel[:, :, None])

        # own shard h0 in permuted layout [128, nwin, H]
        h0_own = np.zeros((shard_pad, H), np.float32)
        vmask = node_at_pos[c] >= 0
        h0_own[vmask] = h0[c * shard + node_at_pos[c, vmask]]
        h0_own = h0_own.reshape(nwin, 128, H).transpose(1, 0, 2)

        mask = (np.arange(128)[:, None] < wns[c][None, :]).astype(np.float32)

        in_maps.append(
            {
                "attrT_aug": np.ascontiguousarray(attrT_aug).astype(ml_dtypes.bfloat16),
                "src_gidx": np.ascontiguousarray(src_gidx),
                "onehot": np.ascontiguousarray(onehot).astype(ml_dtypes.bfloat16),
                "hsrc0": np.ascontiguousarray(hsrc0).astype(ml_dtypes.bfloat16),
                "h0_own": np.ascontiguousarray(h0_own).astype(ml_dtypes.bfloat16),
                "mask": np.ascontiguousarray(mask).astype(ml_dtypes.bfloat16),
                "W_aug": W_aug.astype(ml_dtypes.bfloat16),
                "root_aug": root_aug.astype(ml_dtypes.bfloat16),
                "trans_aug": trans_aug.astype(ml_dtypes.bfloat16),
                "final_aug": final_aug.astype(ml_dtypes.bfloat16),
                "gn": gn,
                "fgn": fgn,
                "ident": ident.astype(ml_dtypes.bfloat16),
                "ones_row": ones_row,
                "ones_col": ones_col,
            }
        )

    shapes = dict(
        N=N, E=E, IN=IN, H=H, EA=EA, T=T, L=L, shard=shard,
        shard_pad=shard_pad, nub=nwin, B=B, block_win=tuple(block_win),
        bw=tuple(int(v) for v in bw), nwin=nwin,
    )
    perms = node_at_pos  # for output unpermute
    return in_maps, shapes, perms


# ------------------------------------------------------------- device build


def build_program(s):
    H, EA, T, L = s["H"], s["EA"], s["T"], s["L"]
    B, nub, nwin = s["B"], s["nub"], s["nwin"]
    shard_pad = s["shard_pad"]
    bw = s["bw"]
    HH = H * H
    HHH = HH // 2
    n_total = shard_pad * NCORES
    n_real = s["N"]

    nc = bacc.Bacc("TRN2", target_bir_lowering=False, debug=False,
                   enable_asserts=False, num_devices=NCORES)

    def din(name, shape, dtype=mdt.float32):
        return nc.dram_tensor(name, shape, dtype, kind="ExternalInput").ap()

    attrT = din("attrT_aug", [EA + 1, B * 128], mdt.bfloat16)
    src_gidx = din("src_gidx", [128, B], mdt.int32)
    onehot_in = din("onehot", [128, B, 128], mdt.bfloat16)
    hsrc0_in = din("hsrc0", [128, B, H], mdt.bfloat16)
    h0_own_in = din("h0_own", [128, nub, H], mdt.bfloat16)
    mask_in = din("mask", [128, nub], mdt.bfloat16)
    W_in = din("W_aug", [L, EA + 1, HH], mdt.bfloat16)
    root_in = din("root_aug", [L, 2 * H + 1, H], mdt.bfloat16)
    trans_in = din("trans_aug", [L, 2 * H + 1, H], mdt.bfloat16)
    final_in = din("final_aug", [2 * H + 1, T], mdt.bfloat16)
    gn_in = din("gn", [L, 1, 4 * H])
    fgn_in = din("fgn", [1, 4 * T])
    ident_in = din("ident", [128, 128], mdt.bfloat16)
    ones_in = din("ones_row", [1, 128])
    onesc_in = din("ones_col", [128, 1])

    out_dram = nc.dram_tensor("out", [shard_pad, T], mdt.float32,
                              kind="ExternalOutput").ap()
    if DEBUG:
        dbg_conv = nc.dram_tensor("dbg_conv", [128, nub, H], mdt.float32,
                                  kind="ExternalOutput").ap()
        dbg_srow = nc.dram_tensor("dbg_srow", [1, 2 * H], mdt.float32,
                                  kind="ExternalOutput").ap()
        dbg_crow = nc.dram_tensor("dbg_crow", [1, 2 * H], mdt.float32,
                                  kind="ExternalOutput").ap()
        dbg_h1 = nc.dram_tensor("dbg_h1", [128, nub, H], mdt.float32,
                                kind="ExternalOutput").ap()
        dbg_msg = nc.dram_tensor("dbg_msg", [128, 8, H], mdt.float32,
                                 kind="ExternalOutput").ap()

    rg = [list(range(NCORES))]

    with tile.TileContext(nc) as tc:
        with (
            tc.tile_pool(name="const", bufs=1) as cpool,
            tc.tile_pool(name="hbuf", bufs=1) as hpool,
            tc.tile_pool(name="ew", bufs=RUN_PAIRS + 2) as ewpool,
            tc.tile_pool(name="tmp", bufs=4) as tmppool,
            tc.tile_pool(name="rows", bufs=10) as rpool,
            tc.tile_pool(name="ps", bufs=1, space="PSUM") as ps,
            tc.tile_pool(name="dram", bufs=1, space="DRAM") as dram,
        ):
            # ---- startup DMAs. First compute needs attrT chunk + W0 (MLP),
            # then hsrc0/onehot chunks (mult/scatter), h0_own (transposes).
            RUNA = 12
            SPL = RUNA * 128
            attrT_sb = cpool.tile([EA + 1, B * 128], mdt.bfloat16, tag="attrT")
            nc.sync.dma_start(attrT_sb[:, 0:SPL], attrT[:, 0:SPL])
            W_l = [cpool.tile([EA + 1, HH], mdt.bfloat16, tag=f"W{li}",
                              name=f"W_{li}") for li in range(L)]
            nc.sync.dma_start(W_l[0][:], W_in[0])
            hA = hpool.tile([128, nub, H], mdt.bfloat16)
            hB = hpool.tile([128, nub, H], mdt.bfloat16)
            nc.sync.dma_start(hA[:], h0_own_in[:])
            ident_sb = cpool.tile([128, 128], mdt.bfloat16, tag="ident")
            nc.sync.dma_start(ident_sb[:], ident_in[:])
            root_l = [cpool.tile([2 * H + 1, H], mdt.bfloat16, tag=f"rw{li}",
                                 name=f"root_{li}") for li in range(L)]
            nc.sync.dma_start(root_l[0][:], root_in[0])

            hsrc0_sb = cpool.tile([128, B, H], mdt.bfloat16, tag="hsrc0")
            nc.scalar.dma_start(hsrc0_sb[:, 0:RUNA, :], hsrc0_in[:, 0:RUNA, :])
            onehot_sb = cpool.tile([128, B, 128], mdt.bfloat16, tag="onehot")
            nc.scalar.dma_start(onehot_sb[:, 0:RUNA, :], onehot_in[:, 0:RUNA, :])
            nc.scalar.dma_start(attrT_sb[:, SPL:], attrT[:, SPL:])
            nc.scalar.dma_start(hsrc0_sb[:, RUNA:, :], hsrc0_in[:, RUNA:, :])

            nc.sync.dma_start(onehot_sb[:, RUNA:, :], onehot_in[:, RUNA:, :])

            def gload(pool, shape, ap, dtype=mdt.float32, tag=None):
                t = pool.tile(shape, dtype, tag=tag, name=tag)
                nc.gpsimd.dma_start(t[:], ap)
                return t

            for li in range(1, L):
                nc.gpsimd.dma_start(W_l[li][:], W_in[li])
                nc.gpsimd.dma_start(root_l[li][:], root_in[li])
            idx_sb = gload(cpool, [128, B], src_gidx[:], mdt.int32, tag="sidx")
            mask_sb = gload(cpool, [128, nub], mask_in[:], mdt.bfloat16,
                            tag="mask")
            final_sb = gload(cpool, [2 * H + 1, T], final_in[:], mdt.bfloat16,
                             tag="finalw")
            onesr_sb = gload(cpool, [1, 128], ones_in[:], tag="onesr")
            onesc_sb = gload(cpool, [128, 1], onesc_in[:], tag="onesc")
            fgn_sb = gload(cpool, [1, 4 * T], fgn_in[:], tag="fgn")
            trans_l = [gload(cpool, [2 * H + 1, H], trans_in[li], mdt.bfloat16,
                             tag=f"tw{li}") for li in range(L)]
            gn_l = [gload(cpool, [1, 4 * H], gn_in[li], tag=f"gn{li}")
                    for li in range(L)]

            # ---- persistent tiles
            hc_all = hpool.tile([128, nub, H], mdt.bfloat16)
            t1_all = hpool.tile([128, nub, H], mdt.float32)
            conv_sb = hpool.tile([128, nub, H], mdt.float32)
            sq_all = hpool.tile([128, nub, H], mdt.float32)
            st2_sb = hpool.tile([128, 2 * H], mdt.float32)
            fst2_sb = hpool.tile([128, 2 * T], mdt.float32)
            fsq_all = hpool.tile([128, nub, T], mdt.float32)
            hsrc_sb = hpool.tile([128, B, H], mdt.bfloat16)
            catT_all = hpool.tile([2 * H + 1, nub, 128], mdt.bfloat16)
            fo_sb = hpool.tile([128, nub, T], mdt.float32)
            y_sb = hpool.tile([128, nub, T], mdt.float32)
            cd_sb = hpool.tile([128, 2 * H], mdt.float32)
            fcd_sb = hpool.tile([128, 2 * T], mdt.float32)
            stats_sb = hpool.tile([1, 2 * H], mdt.float32)
            fstats_sb = hpool.tile([1, 2 * T], mdt.float32)

            nc.vector.memset(catT_all[H : 2 * H, :, :], 0.0)
            nc.vector.memset(catT_all[2 * H : 2 * H + 1, :, :], 1.0)

            # warmup collective: absorbs first-collective latency during
            # the layer-0 edge phase (result unused)
            warm_in = dram.tile([1, 8], mdt.float32)
            warm_out = dram.tile([1, 8], mdt.float32, addr_space="Shared")
            warm2_out = dram.tile([1, 8], mdt.float32, addr_space="Shared")
            wrow = rpool.tile([1, 8], mdt.float32, tag="warm")
            nc.vector.memset(wrow[:], 0.0)
            nc.sync.dma_start(warm_in[:], wrow[:])
            nc.gpsimd.collective_compute(
                "AllReduce", ALU.add, replica_groups=rg,
                ins=[warm_in.opt()], outs=[warm_out.opt()],
            )

            hstage_dram = dram.tile([shard_pad, H], mdt.bfloat16)
            htable_l = [None] + [
                dram.tile([n_total, H], mdt.bfloat16, addr_space="Shared",
                          tag=f"htable{li}", name=f"htable{li}")
                for li in range(1, L)
            ]
            st_in = dram.tile([1, 2 * H], mdt.float32)
            st_out_l = [dram.tile([1, 2 * H], mdt.float32, addr_space="Shared",
                                  tag=f"stout{li}", name=f"stout{li}")
                        for li in range(L)]
            fst_in = dram.tile([1, 2 * T], mdt.float32)
            fst_out = dram.tile([1, 2 * T], mdt.float32, addr_space="Shared")

            hstage_v = hstage_dram[:].rearrange("(u p) f -> p u f", p=128)

            def rstd_row(dstrow, varrow, width, tag):
                """dstrow = 1/sqrt(varrow+EPS) via reciprocal+sqrt+Newton."""
                ve = rpool.tile([1, width], mdt.float32, tag=tag)
                nc.vector.tensor_scalar_add(ve[:], varrow, EPS)
                r2 = rpool.tile([1, width], mdt.float32, tag=tag)
                nc.vector.reciprocal(r2[:], ve[:])
                r0 = rpool.tile([1, width], mdt.float32, tag=tag)
                nc.scalar.activation(r0[:], r2[:], AF.Sqrt)
                t0 = rpool.tile([1, width], mdt.float32, tag=tag)
                nc.vector.tensor_mul(t0[:], r0[:], r0[:])
                nc.vector.tensor_mul(t0[:], t0[:], ve[:])
                nc.vector.scalar_tensor_tensor(
                    t0[:], t0[:], -0.5, r0[:], op0=ALU.mult, op1=ALU.mult
                )
                nc.vector.scalar_tensor_tensor(
                    dstrow, r0[:], 1.5, t0[:], op0=ALU.mult, op1=ALU.add
                )

            def cd_rows(crow, srow, gnrow, width, tag):
                """crow[0:w] = C ; crow[w:2w] = D.

                srow = [sum(c'^2), sum(c')] where c' = conv WITHOUT the folded
                bias rb. gnrow = [w, b, ms, rb].
                var = msq' - ms(2-ms)mean'^2 + (1-ms)^2 (2 rb mean' + rb^2)
                C = w * rstd(var)
                D = b + C*((1-ms)*rb - ms*mean')
                """
                gnw = gnrow[:, 0:width]
                gnb = gnrow[:, width : 2 * width]
                gnms = gnrow[:, 2 * width : 3 * width]
                gnrb = gnrow[:, 3 * width : 4 * width]
                mean = rpool.tile([1, width], mdt.float32, tag=tag)
                nc.vector.tensor_scalar_mul(mean[:], srow[:, width : 2 * width],
                                            1.0 / n_real)
                msq = rpool.tile([1, width], mdt.float32, tag=tag)
                nc.vector.tensor_scalar_mul(msq[:], srow[:, 0:width],
                                            1.0 / n_real)
                mm = rpool.tile([1, width], mdt.float32, tag=tag)
                nc.vector.tensor_mul(mm[:], mean[:], mean[:])
                nc.vector.tensor_mul(mm[:], mm[:], gnms)
                co = rpool.tile([1, width], mdt.float32, tag=tag)
                nc.vector.tensor_scalar(co[:], gnms, -1.0, 2.0, op0=ALU.mult,
                                        op1=ALU.add)
                nc.vector.tensor_mul(mm[:], mm[:], co[:])
                var = rpool.tile([1, width], mdt.float32, tag=tag)
                nc.vector.tensor_sub(var[:], msq[:], mm[:])
                # + (1-ms)^2 * rb * (2*mean' + rb)
                omm = rpool.tile([1, width], mdt.float32, tag=tag)
                nc.vector.tensor_scalar(omm[:], gnms, -1.0, 1.0, op0=ALU.mult,
                                        op1=ALU.add)
                cr = rpool.tile([1, width], mdt.float32, tag=tag)
                nc.vector.scalar_tensor_tensor(cr[:], mean[:], 2.0, gnrb,
                                               op0=ALU.mult, op1=ALU.add)
                nc.vector.tensor_mul(cr[:], cr[:], gnrb)
                nc.vector.tensor_mul(cr[:], cr[:], omm[:])
                nc.vector.tensor_mul(cr[:], cr[:], omm[:])
                nc.vector.tensor_add(var[:], var[:], cr[:])
                rstd = rpool.tile([1, width], mdt.float32, tag=tag)
                rstd_row(rstd[:], var[:], width, tag)
                nc.vector.tensor_mul(crow[:, 0:width], rstd[:], gnw)
                # D = b + C*((1-ms)*rb - ms*mean')
                d1 = rpool.tile([1, width], mdt.float32, tag=tag)
                nc.vector.tensor_mul(d1[:], omm[:], gnrb)
                d2 = rpool.tile([1, width], mdt.float32, tag=tag)
                nc.vector.tensor_mul(d2[:], gnms, mean[:])
                nc.vector.tensor_sub(d1[:], d1[:], d2[:])
                nc.vector.tensor_mul(d1[:], d1[:], crow[:, 0:width])
                nc.vector.tensor_add(crow[:, width : 2 * width], d1[:], gnb)

            def emit_mlp_relu2(li, bp):
                """MLP + Act relu for block pair (2bp, 2bp+1) -> one bf16
                double-wide SBUF ring tile."""
                ew2 = ewpool.tile([128, 2, HH], mdt.bfloat16, tag="ew")
                for j in (0, 1):
                    b = 2 * bp + j
                    pre = ps.tile([128, HH], mdt.float32, tag="pre", bufs=3)
                    a_sl = attrT_sb[:, b * 128 : (b + 1) * 128]
                    nc.tensor.matmul(pre[:, 0:HHH], a_sl, W_l[li][:, 0:HHH],
                                     start=True, stop=True)
                    nc.tensor.matmul(pre[:, HHH:HH], a_sl, W_l[li][:, HHH:HH],
                                     start=True, stop=True)
                    nc.scalar.activation(ew2[:, j, :], pre[:], AF.Relu)
                return ew2

            pending = {}
            hcur, hnxt = hA, hB
            mask_bc3 = mask_sb[:].unsqueeze(2).broadcast_to([128, nub, H])

            for li in range(L):
                hsrc_v = hsrc0_sb if li == 0 else hsrc_sb

                def emit_gather_batch(g0, n):
                    out_ap = (hsrc_sb[:, g0, :] if n == 1
                              else hsrc_sb[:, g0 : g0 + n, :])
                    nc.gpsimd.indirect_dma_start(
                        out=out_ap, out_offset=None,
                        in_=htable_l[li][:],
                        in_offset=bass.IndirectOffsetOnAxis(
                            ap=idx_sb[:, g0 : g0 + n], axis=0),
                    )

                def pool_pair(bp):
                    # pairs whose edge mult runs on GpSimd instead of DVE
                    if li == 0:
                        return bp % 3 != 2 and bp < 21
                    if GB > 1:
                        return bp % 4 == 1 and bp < 40
                    return False

                nbatch = (B + GB - 1) // GB
                GLA = 10  # gather lookahead (batches)
                if li > 0:
                    for g in range(min(GLA, nbatch)):
                        emit_gather_batch(g * GB, min(GB, B - g * GB))

                # ---- edge phase. catT[0:H] = hcur^T comes from the
                # previous layer's pass 2 (layer 0: transposed here).
                assert all(v % 2 == 0 for v in bw)
                if li == 0:
                    for w in range(nwin):
                        tp0 = ps.tile([H, 128], mdt.bfloat16, tag="aggI",
                                      bufs=2, name="tp0")
                        nc.tensor.transpose(tp0[:], hcur[:, w, :], ident_sb[:])
                        nc.scalar.activation(catT_all[0:H, w, :], tp0[:],
                                             AF.Copy)
                b = 0
                # scatters lag the DVE pipeline by one pair; conv copies are
                # emitted when a window's final (stop) scatter retires
                pend_scat = []  # (aggI, msg2, block, j, stop, w)

                def pop_scat():
                    ag, s4t, bb, (jj, kk), stop, ww = pend_scat.pop(0)
                    nc.tensor.matmul(
                        ag[:], onehot_sb[:, bb, :],
                        s4t[:, jj * H : (jj + 1) * H, kk],
                        start=False, stop=stop)
                    if stop:
                        nc.scalar.activation(conv_sb[:, ww, :], ag[:],
                                             AF.Copy)

                for w in range(nwin):
                    aggI = ps.tile([128, H], mdt.float32, tag="aggI", bufs=2)
                    # root matmul opens the accumulation group
                    nc.tensor.matmul(aggI[:], catT_all[:, w, :], root_l[li][:],
                                     start=True, stop=False)
                    for half in range(bw[w] // 2):
                        bp = b // 2
                        if (li, bp) in pending:
                            ew2 = pending.pop((li, bp))
                        else:
                            ew2 = emit_mlp_relu2(li, bp)
                        if li > 0:
                            for bb in (b, b + 1):
                                if bb % GB == 0:
                                    g = bb // GB + GLA
                                    if g < nbatch:
                                        emit_gather_batch(
                                            g * GB, min(GB, B - g * GB))
                        tmp2 = tmppool.tile([128, 2, H, H], mdt.bfloat16,
                                            tag="tmp")
                        if pool_pair(bp):
                            for j in (0, 1):
                                h_bc = (hsrc_v[:, b + j, :].unsqueeze(1)
                                        .broadcast_to([128, H, H]))
                                nc.gpsimd.tensor_tensor(
                                    tmp2[:, j], ew2[:, j, :].rearrange(
                                        "p (o i) -> p o i", o=H, i=H),
                                    h_bc, op=ALU.mult)
                        else:
                            hs2 = (hsrc_v[:, b : b + 2, :].unsqueeze(2)
                                   .broadcast_to([128, 2, H, H]))
                            nc.vector.tensor_tensor(
                                tmp2[:],
                                ew2[:].rearrange("p t (o i) -> p t o i",
                                                 o=H, i=H),
                                hs2, op=ALU.mult)
                        # i-reduction: four 2x tree levels down to 2
                        # partials; the scatter matmuls add the last pair
                        v16 = tmppool.tile([128, 2 * H, 16], mdt.bfloat16,
                                           tag="s1")
                        v8 = tmppool.tile([128, 2 * H, 8], mdt.bfloat16,
                                          tag="s2")
                        v4t = tmppool.tile([128, 2 * H, 4], mdt.bfloat16,
                                           tag="s3")
                        s4 = tmppool.tile([128, 2 * H, 2], mdt.bfloat16,
                                          tag="msg")
                        tv = tmp2[:].rearrange("p t o i -> p (t o) i")
                        with nc.allow_low_precision(reason="msg bf16 i-sum"):
                            nc.vector.tensor_tensor(
                                v16[:], tv[:, :, 0:16], tv[:, :, 16:32],
                                op=ALU.add)
                            nc.vector.tensor_tensor(
                                v8[:], v16[:, :, 0:8], v16[:, :, 8:16],
                                op=ALU.add)
                            nc.vector.tensor_tensor(
                                v4t[:], v8[:, :, 0:4], v8[:, :, 4:8],
                                op=ALU.add)
                            nc.vector.tensor_tensor(
                                s4[:], v4t[:, :, 0:2], v4t[:, :, 2:4],
                                op=ALU.add)
                        last = half == bw[w] // 2 - 1
                        for j in (0, 1):
                            for k in (0, 1):
                                pend_scat.append(
                                    (aggI, s4, b + j, (j, k),
                                     last and j == 1 and k == 1, w))
                        while len(pend_scat) > 4:
                            pop_scat()
                        b += 2
                    if li == 0 and w == 16:
                        nc.gpsimd.collective_compute(
                            "AllReduce", ALU.add, replica_groups=rg,
                            ins=[warm_in.opt()], outs=[warm2_out.opt()],
                        )
                while pend_scat:
                    pop_scat()

                # ---- stats: DVE square + w-reduces + one fp32 ones-matmul
                nc.vector.tensor_mul(sq_all[:], conv_sb[:], conv_sb[:])
                nc.vector.tensor_reduce(
                    st2_sb[:, 0:H], sq_all[:].rearrange("p w f -> p f w"),
                    axis=mybir.AxisListType.X, op=ALU.add)
                nc.vector.tensor_reduce(
                    st2_sb[:, H : 2 * H],
                    conv_sb[:].rearrange("p w f -> p f w"),
                    axis=mybir.AxisListType.X, op=ALU.add)
                smm = ps.tile([1, 2 * H], mdt.float32, tag="aggI", bufs=2)
                nc.tensor.matmul(smm[:], onesc_sb[:], st2_sb[:],
                                 start=True, stop=True)
                nc.scalar.activation(stats_sb[:], smm[:], AF.Copy)

                # ---- stats AllReduce
                nc.sync.dma_start(st_in[:], stats_sb[:])
                st_out = st_out_l[li]
                nc.gpsimd.collective_compute(
                    "AllReduce", ALU.add, replica_groups=rg,
                    ins=[st_in.opt()], outs=[st_out.opt()],
                )

                # ---- next-layer MLP runahead: spread through the AR wait
                # and the pass-2 loops to keep PE/Act busy
                ra_queue = (list(range(min(RUN_PAIRS, B // 2)))
                            if li + 1 < L else [])

                def emit_ra():
                    if ra_queue:
                        rp = ra_queue.pop(0)
                        pending[(li + 1, rp)] = emit_mlp_relu2(li + 1, rp)

                for _ in range(6):
                    emit_ra()

                srow2 = rpool.tile([1, 2 * H], mdt.float32, tag="srow")
                nc.sync.dma_start(srow2[:], st_out[:])
                if DEBUG and li == 0:
                    nc.scalar.dma_start(dbg_conv, conv_sb[:])
                    nc.scalar.dma_start(dbg_srow, srow2[:])

                # ---- C/D rows + broadcast
                crow = rpool.tile([1, 2 * H], mdt.float32, tag="cdrow")
                cd_rows(crow, srow2, gn_l[li][:], H, "nrow")
                if DEBUG and li == 0:
                    nc.scalar.dma_start(dbg_crow, crow[:])
                cd_ps = ps.tile([128, 2 * H], mdt.float32, tag="aggI", bufs=2)
                nc.tensor.matmul(cd_ps[:], onesr_sb[:], crow[:], start=True,
                                 stop=True)
                nc.scalar.activation(cd_sb[:], cd_ps[:], AF.Copy)

                # ---- node pass 2 (batched)
                nc.vector.tensor_tensor(
                    t1_all[:], conv_sb[:],
                    cd_sb[:, 0:H].unsqueeze(1).broadcast_to([128, nub, H]),
                    op=ALU.mult)
                nc.vector.tensor_tensor(
                    t1_all[:], t1_all[:],
                    cd_sb[:, H : 2 * H].unsqueeze(1)
                    .broadcast_to([128, nub, H]),
                    op=ALU.add)
                nc.vector.scalar_tensor_tensor(
                    hc_all[:], t1_all[:], 0.0, hcur[:],
                    op0=ALU.max, op1=ALU.add)
                if li == 2:
                    # fo needs pad rows of hc zeroed (they feed catT H:2H)
                    nc.vector.tensor_tensor(hc_all[:], hc_all[:], mask_bc3,
                                            op=ALU.mult)
                for u in range(nub):
                    if u % 2 == 0:
                        emit_ra()
                    tp2 = ps.tile([H, 128], mdt.bfloat16, tag="aggI", bufs=2)
                    nc.tensor.transpose(tp2[:], hc_all[:, u, :], ident_sb[:])
                    nc.scalar.activation(catT_all[H : 2 * H, u, :],
                                         tp2[:], AF.Copy)
                for u in range(nub):
                    if u % 2 == 1:
                        emit_ra()
                    tr = ps.tile([128, H], mdt.float32, tag="aggI", bufs=2)
                    nc.tensor.matmul(tr[:], catT_all[:, u, :], trans_l[li][:],
                                     start=True, stop=True)
                    # masked relu keeps pad rows zero (stats + final need it)
                    nc.vector.scalar_tensor_tensor(
                        hnxt[:, u, :], tr[:], 0.0,
                        mask_sb[:, u : u + 1].broadcast_to([128, H]),
                        op0=ALU.max, op1=ALU.mult)
                    if li + 1 < L:
                        # stage this window for the AllGather right away
                        nc.sync.dma_start(hstage_v[:, u, :], hnxt[:, u, :])
                # AllGather starts here; tp3 loop + runahead overlap it
                if li + 1 < L:
                    nc.gpsimd.collective_compute(
                        "AllGather", ALU.bypass, replica_groups=rg,
                        ins=[hstage_dram.opt()], outs=[htable_l[li + 1].opt()],
                    )
                # next layer's catT[0:H] = hnxt^T (overwrites hcur^T)
                for u in range(nub):
                    if u % 2 == 0:
                        emit_ra()
                    tp3 = ps.tile([H, 128], mdt.bfloat16, tag="aggI", bufs=2)
                    nc.tensor.transpose(tp3[:], hnxt[:, u, :], ident_sb[:])
                    nc.scalar.activation(catT_all[0:H, u, :], tp3[:], AF.Copy)
                    if li == 2:
                        f_ps = ps.tile([128, T], mdt.float32, tag="aggI",
                                       bufs=2)
                        nc.tensor.matmul(f_ps[:], catT_all[:, u, :],
                                         final_sb[:], start=True, stop=True)
                        if u % 2 == 0:
                            nc.scalar.activation(fo_sb[:, u, :], f_ps[:],
                                                 AF.Copy)
                        else:
                            nc.vector.tensor_copy(fo_sb[:, u, :], f_ps[:])

                # flush remaining runahead before the AllGather
                while ra_queue:
                    emit_ra()

                hcur, hnxt = hnxt, hcur

            # ============ final stats (fo computed in layer-2 pass 2) ======
            nc.vector.tensor_mul(fsq_all[:], fo_sb[:], fo_sb[:])
            nc.vector.tensor_reduce(
                fst2_sb[:, 0:T], fsq_all[:].rearrange("p w f -> p f w"),
                axis=mybir.AxisListType.X, op=ALU.add)
            nc.vector.tensor_reduce(
                fst2_sb[:, T : 2 * T], fo_sb[:].rearrange("p w f -> p f w"),
                axis=mybir.AxisListType.X, op=ALU.add)
            fsmm = ps.tile([1, 2 * T], mdt.float32, tag="aggI", bufs=2)
            nc.tensor.matmul(fsmm[:], onesc_sb[:], fst2_sb[:],
                             start=True, stop=True)
            nc.scalar.activation(fstats_sb[:], fsmm[:], AF.Copy)

            nc.sync.dma_start(fst_in[:], fstats_sb[:])
            nc.gpsimd.collective_compute(
                "AllReduce", ALU.add, replica_groups=rg,
                ins=[fst_in.opt()], outs=[fst_out.opt()],
            )
            fsrow2 = rpool.tile([1, 2 * T], mdt.float32, tag="fsrow")
            nc.sync.dma_start(fsrow2[:], fst_out[:])

            fcrow = rpool.tile([1, 2 * T], mdt.float32, tag="fcdrow")
            cd_rows(fcrow, fsrow2, fgn_sb[:], T, "frow")
            fcd_ps = ps.tile([128, 2 * T], mdt.float32, tag="aggI", bufs=2)
            nc.tensor.matmul(fcd_ps[:], onesr_sb[:], fcrow[:], start=True,
                             stop=True)
            nc.scalar.activation(fcd_sb[:], fcd_ps[:], AF.Copy)

            nc.vector.tensor_tensor(
                y_sb[:], fo_sb[:],
                fcd_sb[:, 0:T].unsqueeze(1).broadcast_to([128, nub, T]),
                op=ALU.mult)
            nc.vector.tensor_tensor(
                y_sb[:], y_sb[:],
                fcd_sb[:, T : 2 * T].unsqueeze(1).broadcast_to([128, nub, T]),
                op=ALU.add)
            nc.vector.tensor_scalar_max(y_sb[:], y_sb[:], 0.0)
            out_v = out_dram.rearrange("(u p) f -> p u f", p=128)
            nc.sync.dma_start(out_v, y_sb[:])

    nc.compile()
    return nc


# ------------------------------------------------------------------ driver

_CACHE = {}


def kernel(**inputs) -> np.ndarray:
    in_maps, s, node_at_pos = prep_inputs(inputs)
    key = (s["N"], s["E"], s["B"], s["block_win"])
    if key not in _CACHE:
        _CACHE[key] = build_program(s)
    nc = _CACHE[key]

    from concourse.bass_utils import run_bass_kernel_spmd

    res = run_bass_kernel_spmd(nc, in_maps, core_ids=list(range(NCORES)))
    shard, T, N = s["shard"], s["T"], s["N"]
    out = np.empty((N, T), np.float32)
    for c in range(NCORES):
        rows = res.results[c]["out"]
        vmask = node_at_pos[c] >= 0
        out[c * shard + node_at_pos[c, vmask]] = rows[vmask]
    return out.astype(np.float32)
